# revision 48
# baseline (speedup 1.0000x reference)
"""Trainium2 Bass kernel for an RNN-T style joint network MLP.

  out[b,t,u,o] = tanh(enc[b,t,:] @ W1[:512] + dec[b,u,:] @ W1[512:] + b1) @ W2 + b2

Shapes: enc (8, 256, 512), dec (8, 64, 512), W1 (1024, 1024), b1 (1024,),
W2 (1024, 128), b2 (128,), out (8, 256, 64, 128), all float32.

Sharding: data-parallel over batch — one batch element per NeuronCore, no
collectives.  The kernel is elementwise-bound: 16.8M hidden elements per core
need a broadcast-add (DVE tensor_scalar, 2x bf16) and a tanh (ACT, 1
elem/cycle/lane).  Steady state balances ACT ~= DVE ~= 119us busy:
  - ACT: big per-block tanh ops + a few PSUM evacuations + head e_proj evacs
  - DVE: all 512 broadcast-adds + most PSUM evacuations (+b2)
  - PE:  enc/dec projections, then the main GEMM (N=512 per u-pair)
Head (the ~2.4MB input load is DMA-bandwidth-bound at ~290GB/s and gates
everything): W1 is host-swizzled hc-major and loaded in per-hc slices so the
first h-chunk's GEMM starts ~1.5us after DMA start; the first 3 u-blocks'
adds/tanh are emitted hc-major ACROSS blocks (and hc5-7's first GEMMs
emitted inside that stream) so the strict-FIFO engine queues process work in
W1-slice-arrival order instead of stalling on a late slice; dma_start issues
are serialized on the SP queue in priority order (ring FIFO = issue order)
with only tiny tensors on gpsimd/SWDGE; dummy PE warm-up matmuls lift the
HAM clock gate to 2.4GHz before the real GEMM.  Tail: tapered final blocks
(UB=2) with split tanh and per-u evac+store shorten the drain.  Each block's
PSUM evacuation is emitted one block late so it never heads the DVE queue
before the next block's adds.

A custom deg-7 odd-polynomial tanh DVE op (TANH7_ANT, 8 chained ALU
slices, registered via the documented dve_ops.OPS extension point) takes
over hc7's tanh on four late blocks -- it fills DVE's end-of-stream idle
window and shortens the ACT stream by ~3us; more than that and DVE becomes
the pacer (1.25ns/elem on DVE buys 0.88ns/elem of ACT).

Measured (8 axon trn2 cores): ~142.6-143.3us HW exec (from 151.7us
baseline), rel err 8.0e-3 vs fp32 reference (3.6e-3 bf16 datapath + the
poly approximation on 4/16 blocks' hc7).  Balanced: ACT 121.3us busy
(111.7us tanh stream at the 1 elem/cycle/lane floor + evacs), DVE 120.8us.

Tried and rejected (measured): GPSIMD broadcast-adds -- gpsimd shares its
SBUF port with the DVE and concurrent ops serialize (a DVE op stalls for
the full duration of an overlapping gpsimd op) while gpsimd's rate is 2.4x
worse; UB=6 blocks -- PSUM forces bufs=2 and the pipeline serializes;
bf16 output -- stores are descriptor-latency-bound, no tail gain.
"""

import os
import numpy as np
import ml_dtypes

B, T, U, D, H, O = 8, 256, 64, 512, 1024, 128
NCORES = 8
HC = H // 128     # 8 h-chunks

# u-block sizes (pipeline granularity); tapered tail for a short drain
UBS = [4] * 15 + [2, 2]
# blocks whose PSUM evacuation (+b2) runs on ACT instead of DVE (balance knob;
# early blocks, where DVE is still ramping the add pipeline)
ACT_EVAC_BLOCKS = frozenset({0, 1, 3, 6, 8, 11})
N_HEAD_BLOCKS = 3   # blocks emitted hc-major (adds/tanh follow W1-slice arrival)
# h-chunks whose e_proj PSUM evac runs on ACT (rest on DVE)
ACT_EEVAC_HCS = frozenset(range(5))
N_WARMUP_MM = 4   # dummy matmuls to lift the PE HAM clock gate before the GEMM

# custom DVE deg-7 odd-poly tanh: fit on [-3.6,3.6], weight exp(-x^2/4s^2)+0.1
TANH7_C = (0.9555391354960743, -0.20140714151381847,
           0.023106368613611413, -0.0009152143704840363)
# blocks whose hc7 tanh runs as the custom op on DVE (fills DVE's measured
# end-of-stream idle window while shortening the ACT stream)
DVE_TANH_BLOCKS = frozenset({11, 12, 13, 14})

_CACHE = {}
LAST_RESULT = None  # BassKernelResults from the most recent run (for profiling)


def _register_tanh7():
    """Register the TANH7_ANT custom DVE op (documented extension point:
    dve_ops.OPS + name->row map).  Idempotent."""
    import concourse.dve_ops as dve_ops
    from concourse.dve_spec import Spec, Src0, C0, C1, C2, C3, sq, _spill_c3_to_src1

    for op in dve_ops.OPS:
        if op.name == "TANH7_ANT":
            return op

    x = Src0
    s = sq(x)
    body = _spill_c3_to_src1(x * (C0 + s * (C1 + s * (C2 + s * C3))))

    def ref(in0, in1, s0, s1, imm2):
        xf = in0.astype(np.float32)
        c7 = in1.reshape(in1.shape[0], -1)[:, :1].astype(np.float32)
        ss = xf * xf
        return (xf * (s0 + ss * (s1 + ss * (imm2 + ss * c7)))).astype(np.float32)

    op = dve_ops.DveOp(
        "TANH7_ANT",
        Spec(body=body, reference=ref),
        subdim=False,
        uops_sha={"v3": "fabb8cce46cda8f0", "v4": "fc4459a23b42cb8f"},
    )
    dve_ops.OPS.append(op)
    dve_ops.CUSTOM_DVE_SPECS[op.name] = op.spec
    dve_ops._SUB_OPCODE_FOR_NAME[op.name] = (
        dve_ops._CUSTOM_DVE_ROW_BASE + len(dve_ops.OPS) - 1)
    return op


def _build_program():
    from concourse import bacc, tile
    import concourse.mybir as mybir

    dt = mybir.dt
    f32, bf16 = dt.float32, dt.bfloat16
    Act = mybir.ActivationFunctionType

    TANH7 = _register_tanh7()

    nc = bacc.Bacc("TRN2", target_bir_lowering=False, debug=False)

    # host-side pre-swizzled layouts: every dram row maps to one partition row
    # with a 2KB contiguous extent (fewest DMA descriptors)
    encTr = nc.dram_tensor("encTr", [128, 4 * T], bf16, kind="ExternalInput").ap()
    decTr = nc.dram_tensor("decTr", [128, 4 * U], bf16, kind="ExternalInput").ap()
    # W1 host-swizzled hc-major: W1e_r[p, hc*512 + dc*128 + j] = W1[dc*128+p, hc*128+j]
    # so each per-hc dma_start (128KB) unblocks that h-chunk's first GEMM.
    W1e = nc.dram_tensor("W1e", [128, HC * 512], bf16, kind="ExternalInput").ap()
    W1d = nc.dram_tensor("W1d", [128, HC * 512], bf16, kind="ExternalInput").ap()
    W2r = nc.dram_tensor("W2r", [128, HC * O], bf16, kind="ExternalInput").ap()
    b1r = nc.dram_tensor("b1r", [128, HC], f32, kind="ExternalInput").ap()
    b2c = nc.dram_tensor("b2c", [O, 1], f32, kind="ExternalInput").ap()
    outT = nc.dram_tensor("outT", [O, U, T], f32, kind="ExternalOutput").ap()

    with tile.TileContext(nc) as tc:
        with tc.tile_pool(name="persist", bufs=1) as persist, \
             tc.tile_pool(name="sums", bufs=4) as sums_pool, \
             tc.tile_pool(name="tanhp", bufs=4) as tanh_pool, \
             tc.tile_pool(name="outsb", bufs=3) as out_pool, \
             tc.tile_pool(name="hpsum", bufs=2, space="PSUM") as hpsum_pool, \
             tc.tile_pool(name="psum", bufs=3, space="PSUM") as psum_pool:

            w1e_sb = persist.tile([128, HC * 512], bf16, tag="w1e")
            w1d_sb = persist.tile([128, HC * 512], bf16, tag="w1d")
            encT_sb = persist.tile([128, 4 * T], bf16, tag="encT")
            decT_sb = persist.tile([128, 4 * U], bf16, tag="decT")
            w2_sb = persist.tile([128, HC * O], bf16, tag="w2")
            b1_sb = persist.tile([128, HC], f32, tag="b1")
            b2_sb = persist.tile([128, 1], f32, tag="b2")
            e_sb = persist.tile([128, HC * T], bf16, tag="eproj")
            bias_sb = persist.tile([128, HC * U], f32, tag="bias")
            scr_sb = persist.tile([128, 512], bf16, tag="scratch")
            c7_sb = persist.tile([128, 1], f32, tag="c7")

            # ---- PE warm-up: dummy matmuls on scratch data keep the PE busy
            # from t~7us so the HAM clock gate is at 2.4GHz when the real
            # GEMM starts (saves ~3us of half-clock matmuls at the head).
            nc.vector.memset(scr_sb[:], 0.0)
            nc.vector.memset(c7_sb[:], float(TANH7_C[3]))
            pw = hpsum_pool.tile([128, 512], f32, tag="ps", name="warm")
            for i in range(N_WARMUP_MM):
                nc.tensor.matmul(pw[:], lhsT=scr_sb[:, 0:128], rhs=scr_sb[:],
                                 start=True, stop=True)

            # ---- loads: DMA descriptors spray across all 16 rings, so the
            # head is bandwidth-bound (~2.4MB at ~290GB/s = 8.5us).  W1 is
            # loaded hc-sliced (host-swizzled) so hc0's 256KB lands in ~1us
            # and the first GEMM pipelines with the rest of the load.  Issues
            # come from three engine queues (SP/ACT HWDGE + gpsimd SWDGE) so
            # their ~0.65us per-call issue cost is paid in parallel.
            # hc0 slices first (256KB unblocks the first h-chunk's GEMM in
            # ~1.5us), then growing slices pipeline behind.  The Scalar queue
            # issues nothing so e_proj evacs aren't stuck behind DIRECT2Ds.
            # ring FIFO order == issue-time order across queues, so the big
            # trailing loads must be issued strictly after the early slices;
            # gpsimd (SWDGE) only issues tiny tensors that can't congest.
            nc.sync.dma_start(encT_sb[:], encTr[:, :])
            nc.sync.dma_start(w1e_sb[:, 0:512], W1e[:, 0:512])
            nc.sync.dma_start(w1d_sb[:, 0:512], W1d[:, 0:512])
            nc.sync.dma_start(w1e_sb[:, 512:1024], W1e[:, 512:1024])
            nc.sync.dma_start(w1d_sb[:, 512:1024], W1d[:, 512:1024])
            nc.sync.dma_start(w1e_sb[:, 1024:2048], W1e[:, 1024:2048])
            nc.sync.dma_start(w1d_sb[:, 1024:2048], W1d[:, 1024:2048])
            nc.sync.dma_start(w1e_sb[:, 2048:3072], W1e[:, 2048:3072])
            nc.sync.dma_start(w1d_sb[:, 2048:3072], W1d[:, 2048:3072])
            nc.sync.dma_start(w1e_sb[:, 3072:8 * 512], W1e[:, 3072:8 * 512])
            nc.sync.dma_start(w1d_sb[:, 3072:8 * 512], W1d[:, 3072:8 * 512])
            nc.sync.dma_start(w2_sb[:], W2r[:, :])
            nc.gpsimd.dma_start(b1_sb[:], b1r[:, :])
            nc.gpsimd.dma_start(decT_sb[:], decTr[:, :])
            nc.gpsimd.dma_start(b2_sb[:], b2c[:, :])

            # ---- first GEMMs, interleaved per h-chunk so downstream adds can
            # start on hc0 while hc1.. are still multiplying.
            # enc: e_projT[h,t] = sum_d W_enc[d,h]*encT[d,t]
            # dec: bias[h,u] = sum_d W_dec[d,h]*decT[d,u] + b1 (evac on DVE)
            def first_gemm(hc):
                pe = hpsum_pool.tile([128, T], f32, tag="ps", name=f"pe{hc}")
                for dc in range(4):
                    nc.tensor.matmul(
                        pe[:],
                        lhsT=w1e_sb[:, hc * 512 + dc * 128: hc * 512 + dc * 128 + 128],
                        rhs=encT_sb[:, dc * T:(dc + 1) * T],
                        start=(dc == 0), stop=(dc == 3),
                    )
                if hc in ACT_EEVAC_HCS:
                    nc.scalar.activation(e_sb[:, hc * T:(hc + 1) * T], pe[:],
                                         Act.Identity)
                else:
                    nc.vector.tensor_copy(e_sb[:, hc * T:(hc + 1) * T], pe[:])

                pd = hpsum_pool.tile([128, U], f32, tag="ps", name=f"pd{hc}")
                for dc in range(4):
                    nc.tensor.matmul(
                        pd[:],
                        lhsT=w1d_sb[:, hc * 512 + dc * 128: hc * 512 + dc * 128 + 128],
                        rhs=decT_sb[:, dc * U:(dc + 1) * U],
                        start=(dc == 0), stop=(dc == 3),
                    )
                nc.vector.tensor_scalar_add(bias_sb[:, hc * U:(hc + 1) * U],
                                            pd[:], b1_sb[:, hc:hc + 1])

            # hc0-4 up front (their W1 slices land first); hc5-7 are emitted
            # inside the head superblock right before their adds, so neither
            # the ACT nor the DVE FIFO blocks on a late W1 slice while ready
            # work waits behind it.
            for hc in range(5):
                first_gemm(hc)

            # ---- main pipeline over u-blocks ----
            # sum/tanh layout per block: [hc][u][t] (hc-major); the main GEMM
            # runs N=512 per u-pair into one 1-2 bank PSUM tile.
            #
            # Head superblock: the adds/tanh of the first N_HEAD_BLOCKS
            # blocks are emitted hc-major ACROSS blocks, so the strict-FIFO
            # engine queues process work in W1-slice-arrival order instead of
            # stalling on a not-yet-loaded h-chunk while ready work waits.
            u0s = [sum(UBS[:b]) for b in range(len(UBS))]
            hn = N_HEAD_BLOCKS
            head_sum = [sums_pool.tile([128, UBS[b] * 2048], bf16, tag="sum",
                                       name=f"hsum{b}") for b in range(hn)]
            head_tanh = [tanh_pool.tile([128, UBS[b] * 2048], bf16, tag="tanh",
                                        name=f"htanh{b}") for b in range(hn)]
            for hc in range(HC):
                if hc >= 5:
                    first_gemm(hc)
                for b in range(hn):
                    ub = UBS[b]
                    hcw = ub * T
                    for ul in range(ub):
                        nc.vector.tensor_scalar_add(
                            head_sum[b][:, hc * hcw + ul * T: hc * hcw + ul * T + T],
                            e_sb[:, hc * T:(hc + 1) * T],
                            bias_sb[:, hc * U + u0s[b] + ul: hc * U + u0s[b] + ul + 1],
                        )
                # single-hc tanh ops for hc0/hc1 (earliest possible ACT
                # start), 2-hc groups after
                if hc < 2:
                    for b in range(hn):
                        hcw = UBS[b] * T
                        nc.scalar.activation(
                            head_tanh[b][:, hc * hcw:(hc + 1) * hcw],
                            head_sum[b][:, hc * hcw:(hc + 1) * hcw], Act.Tanh)
                elif hc % 2 == 1:
                    for b in range(hn):
                        hcw = UBS[b] * T
                        nc.scalar.activation(
                            head_tanh[b][:, (hc - 1) * hcw:(hc + 1) * hcw],
                            head_sum[b][:, (hc - 1) * hcw:(hc + 1) * hcw],
                            Act.Tanh)

            pending = []
            for blk, ub in enumerate(UBS):
                bw = ub * 2048      # block free width
                hcw = ub * T        # per-(block, hc) width
                u0 = u0s[blk]

                if blk < hn:
                    sum_sb, tanh_sb = head_sum[blk], head_tanh[blk]
                else:
                    sum_sb = sums_pool.tile([128, bw], bf16, tag="sum")
                    for hc in range(HC):
                        for ul in range(ub):
                            nc.vector.tensor_scalar_add(
                                sum_sb[:, hc * hcw + ul * T: hc * hcw + ul * T + T],
                                e_sb[:, hc * T:(hc + 1) * T],
                                bias_sb[:, hc * U + u0 + ul: hc * U + u0 + ul + 1],
                            )

                    tanh_sb = tanh_pool.tile([128, bw], bf16, tag="tanh")
                    if blk in DVE_TANH_BLOCKS:
                        # hc7's tanh as the custom deg-7 poly op on DVE --
                        # fills DVE's end-of-stream idle, shortens ACT's
                        nc.scalar.activation(tanh_sb[:, 0:7 * hcw],
                                             sum_sb[:, 0:7 * hcw], Act.Tanh)
                        nc.vector._custom_dve(
                            TANH7,
                            out=tanh_sb[:, 7 * hcw:8 * hcw],
                            in0=sum_sb[:, 7 * hcw:8 * hcw],
                            in1=c7_sb[:, 0:1],
                            s0=float(TANH7_C[0]), s1=float(TANH7_C[1]),
                            imm2=float(TANH7_C[2]),
                        )
                    else:
                        # split the tail blocks' tanh so the PE chases the drain
                        nsplit = 2 if blk >= len(UBS) - 2 else 1
                        for q in range(nsplit):
                            nc.scalar.activation(
                                tanh_sb[:, q * bw // nsplit:(q + 1) * bw // nsplit],
                                sum_sb[:, q * bw // nsplit:(q + 1) * bw // nsplit],
                                Act.Tanh)

                pw = min(2, ub) * T      # GEMM free width per psum slot
                npair = max(1, ub // 2)
                po = psum_pool.tile([128, npair * pw], f32, tag="ps",
                                    name=f"po{blk}")
                for hc in range(HC):  # hc outer: W2 chunk stays stationary
                    for p in range(npair):
                        nc.tensor.matmul(
                            po[:, p * pw:(p + 1) * pw],
                            lhsT=w2_sb[:, hc * O:(hc + 1) * O],
                            rhs=tanh_sb[:, hc * hcw + p * pw: hc * hcw + (p + 1) * pw],
                            start=(hc == 0), stop=(hc == HC - 1),
                        )

                # defer this block's evacuation by one block: the next
                # block's adds must enter the DVE FIFO before this evac,
                # which waits on this block's GEMM (else the queue head
                # stalls while ready adds sit behind it)
                pending.append((blk, ub, u0, po))
                if blk < len(UBS) - 1 and len(pending) < 2:
                    continue
                while pending:
                    blk, ub, u0, po = pending.pop(0)
                    out_sb = out_pool.tile([128, ub * T], f32, tag="osb")
                    if blk == len(UBS) - 1:
                        # final block: evac+store per u so the drain is short
                        for j in range(ub):
                            nc.vector.tensor_scalar_add(
                                out_sb[:, j * T:(j + 1) * T],
                                po[:, j * T:(j + 1) * T], b2_sb[:, 0:1])
                            nc.sync.dma_start(outT[:, u0 + j:u0 + j + 1, :],
                                              out_sb[:, j * T:(j + 1) * T])
                    elif blk in ACT_EVAC_BLOCKS:
                        nc.scalar.activation(out_sb[:], po[:], Act.Identity,
                                             bias=b2_sb[:, 0:1])
                        nc.sync.dma_start(outT[:, u0:u0 + ub, :], out_sb[:])
                    else:
                        nc.vector.tensor_scalar_add(out_sb[:], po[:],
                                                    b2_sb[:, 0:1])
                        nc.sync.dma_start(outT[:, u0:u0 + ub, :], out_sb[:])

    nc.compile()
    return nc


def _host_inputs(enc_i, dec_i, b1r, b2c):
    """Per-core input map with pre-swizzled layouts (2KB/partition rows)."""
    bf = ml_dtypes.bfloat16
    # encTr[p, c*T+t] = enc[t, c*128+p]
    encT = np.ascontiguousarray(enc_i.T.astype(bf))          # [512, 256]
    encTr = np.ascontiguousarray(
        encT.reshape(4, 128, T).transpose(1, 0, 2).reshape(128, 4 * T))
    decT = np.ascontiguousarray(dec_i.T.astype(bf))          # [512, 64]
    decTr = np.ascontiguousarray(
        decT.reshape(4, 128, U).transpose(1, 0, 2).reshape(128, 4 * U))
    return {"encTr": encTr, "decTr": decTr, "b1r": b1r, "b2c": b2c}


def _host_weights(W1, W2, bf):
    """W1e/W1d hc-major swizzles + W2r."""
    # W1e[p, hc*512 + dc*128 + j] = W1[dc*128 + p, hc*128 + j]
    We = W1[:D].astype(bf).reshape(4, 128, HC, 128)
    W1e = np.ascontiguousarray(We.transpose(1, 2, 0, 3).reshape(128, HC * 512))
    Wd = W1[D:].astype(bf).reshape(4, 128, HC, 128)
    W1d = np.ascontiguousarray(Wd.transpose(1, 2, 0, 3).reshape(128, HC * 512))
    W2r = np.ascontiguousarray(
        W2.astype(bf).reshape(HC, 128, O).transpose(1, 0, 2).reshape(128, HC * O))
    return W1e, W1d, W2r


def kernel(encoder_state, decoder_state, W1, b1, W2, b2):
    from concourse.bass_utils import run_bass_kernel_spmd
    global LAST_RESULT

    if "nc" not in _CACHE:
        _CACHE["nc"] = _build_program()
    nc = _CACHE["nc"]

    encoder_state = np.asarray(encoder_state, dtype=np.float32)
    decoder_state = np.asarray(decoder_state, dtype=np.float32)
    W1 = np.asarray(W1, dtype=np.float32)
    b1 = np.asarray(b1, dtype=np.float32)
    W2 = np.asarray(W2, dtype=np.float32)
    b2 = np.asarray(b2, dtype=np.float32)

    bf = ml_dtypes.bfloat16
    W1e, W1d, W2r = _host_weights(W1, W2, bf)
    b1r = np.ascontiguousarray(b1.reshape(HC, 128).T)  # [128, 8]
    b2c = np.ascontiguousarray(b2.reshape(O, 1))

    in_maps = []
    for i in range(NCORES):
        m = _host_inputs(encoder_state[i], decoder_state[i], b1r, b2c)
        m.update({"W1e": W1e, "W1d": W1d, "W2r": W2r})
        in_maps.append(m)

    trace = bool(int(os.environ.get("KERNEL_TRACE", "0")))
    res = run_bass_kernel_spmd(nc, in_maps, list(range(NCORES)), trace=trace)
    LAST_RESULT = res

    # gather: outT[core] is [O, U, T] -> out[b, t, u, o]
    out = np.empty((B, T, U, O), dtype=np.float32)
    for i in range(NCORES):
        out[i] = res.results[i]["outT"].transpose(2, 1, 0)
    return out
